# revision 3
# baseline (speedup 1.0000x reference)
"""Bass/Trainium2 kernel for DropConnect (training path, Wstd != 0).

Z[b,o] = sum_i X[b,i] * W[i,o] * Werr[loc_id[b],i,o] + bias[o] * Berr[loc_id[b],o]

Strategy (8 NeuronCores, data-parallel over batch):
  - each core handles 16 samples; W/bias and the Werr/Berr pools are replicated
  - per sample, the 1MB Werr[loc] slab is gathered on-device with one indirect
    DMA: Werr viewed as [128000, 2048] macro-rows, dest partition p pulls the
    contiguous 8KB macro-row loc*128+p (i.e. input rows i=4p..4p+3).
    ALL 16 gathers are issued back-to-back upfront on gpsimd (descriptor
    generation ~1.05us each) so the 16 DMA engines never starve; gpsimd is
    then free to help VectorE with the elementwise multiplies.
  - the W*Werr product is written as bf16: VectorE alone for early samples,
    Vector+GpSimd column-split once gpsimd has finished issuing gathers
  - TensorE contracts with X (bf16): 4 matmuls of [128,1]x[128,512] into a
    [1,512] PSUM tile, plus a 5th matmul (one-hot [16,1] against the
    precomputed bias*Berr[loc] [16,512] bf16 tile) to add the bias
  - ScalarE copies each sample's PSUM row into a [1, 8192] staging tile;
    output ships in two halves
"""

import sys

sys.path.insert(0, "/opt/trn_rl_repo")

import numpy as np

B, IN, OUT, POOL, NCORES = 128, 512, 512, 1000, 8
BL = B // NCORES  # samples per core
WT_COLS = 4 * OUT  # 2048: one macro-row = 4 input rows of W/Werr

_CACHE = {}


def _build(pool_entries=POOL):
    import concourse.bass as bass
    import concourse.mybir as mybir
    import concourse.tile as tile
    from concourse import bacc

    f32, i32, bf16 = mybir.dt.float32, mybir.dt.int32, mybir.dt.bfloat16

    nc = bacc.Bacc("TRN2", debug=False)
    werr = nc.dram_tensor(
        "Werr", [pool_entries * 128, WT_COLS], f32, kind="ExternalInput"
    )
    berr = nc.dram_tensor("Berr", [pool_entries, OUT], f32, kind="ExternalInput")
    wr = nc.dram_tensor("Wr", [128, WT_COLS], f32, kind="ExternalInput")
    xt = nc.dram_tensor("Xt", [128, BL * 4], f32, kind="ExternalInput")
    idx = nc.dram_tensor("idx", [128, BL], i32, kind="ExternalInput")
    loc = nc.dram_tensor("loc", [BL, 1], i32, kind="ExternalInput")
    bias16 = nc.dram_tensor("bias16", [BL, OUT], f32, kind="ExternalInput")
    eye16 = nc.dram_tensor("eye16", [BL, BL], f32, kind="ExternalInput")
    z = nc.dram_tensor("Z", [1, BL * OUT], f32, kind="ExternalOutput")

    # samples whose elementwise multiply is split Vector/GpSimd: gpsimd is
    # busy with descriptor generation for roughly the first 8 samples' worth
    # of pipeline, so only late samples get the split
    SPLIT_FROM = 8
    VSPLIT = 1024

    with tile.TileContext(nc) as tc:
        with (
            tc.tile_pool(name="const", bufs=1) as cpool,
            tc.tile_pool(name="wts", bufs=14) as wpool,
            tc.tile_pool(name="prod", bufs=4) as ptpool,
            tc.tile_pool(name="ps", bufs=8, space="PSUM") as ppool,
        ):
            # idx first: the Werr gathers are gated only on this tiny load
            idx_sb = cpool.tile([128, BL], i32)
            nc.sync.dma_start(idx_sb[:], idx.ap())
            loc_sb = cpool.tile([BL, 1], i32)
            nc.sync.dma_start(loc_sb[:], loc.ap())

            # bias*Berr path first on gpsimd: sample 0's bias-matmul needs it
            berr_sb = cpool.tile([BL, OUT], f32)
            nc.gpsimd.indirect_dma_start(
                out=berr_sb[:],
                out_offset=None,
                in_=berr.ap(),
                in_offset=bass.IndirectOffsetOnAxis(ap=loc_sb[:, :1], axis=0),
            )

            # all 16 slab gathers enqueue back-to-back on gpsimd
            wts = []
            for b in range(BL):
                wt = wpool.tile([128, WT_COLS], f32, tag="wt")
                nc.gpsimd.indirect_dma_start(
                    out=wt[:],
                    out_offset=None,
                    in_=werr.ap(),
                    in_offset=bass.IndirectOffsetOnAxis(
                        ap=idx_sb[:, b : b + 1], axis=0
                    ),
                )
                wts.append(wt)

            wr_sb = cpool.tile([128, WT_COLS], f32)
            nc.sync.dma_start(wr_sb[:], wr.ap())
            xt_sb = cpool.tile([128, BL * 4], f32)
            nc.sync.dma_start(xt_sb[:], xt.ap())
            bias_sb = cpool.tile([BL, OUT], f32)
            nc.sync.dma_start(bias_sb[:], bias16.ap())
            eye_sb = cpool.tile([BL, BL], f32)
            nc.sync.dma_start(eye_sb[:], eye16.ap())
            zstage = cpool.tile([1, BL * OUT], f32)

            # bf16 copies of the small stationary matmul operands
            xtr_sb = cpool.tile([128, BL * 4], bf16)
            nc.vector.tensor_copy(xtr_sb[:], xt_sb[:])
            eyer_sb = cpool.tile([BL, BL], bf16)
            nc.vector.tensor_copy(eyer_sb[:], eye_sb[:])
            memb_sb = cpool.tile([BL, OUT], bf16)
            nc.vector.tensor_mul(memb_sb[:], berr_sb[:], bias_sb[:])

            for b in range(BL):
                wt = wts[b]
                pt = ptpool.tile([128, WT_COLS], bf16, tag="pt")
                if b < SPLIT_FROM:
                    nc.vector.tensor_mul(pt[:], wt[:], wr_sb[:])
                else:
                    nc.vector.tensor_mul(
                        pt[:, :VSPLIT], wt[:, :VSPLIT], wr_sb[:, :VSPLIT]
                    )
                    nc.gpsimd.tensor_mul(
                        pt[:, VSPLIT:], wt[:, VSPLIT:], wr_sb[:, VSPLIT:]
                    )
                ps = ppool.tile([1, OUT], f32, tag="ps")
                for j in range(4):
                    nc.tensor.matmul(
                        out=ps[:],
                        lhsT=xtr_sb[:, 4 * b + j : 4 * b + j + 1],
                        rhs=pt[:, j * OUT : (j + 1) * OUT],
                        start=(j == 0),
                        stop=False,
                    )
                nc.tensor.matmul(
                    out=ps[:],
                    lhsT=eyer_sb[:, b : b + 1],
                    rhs=memb_sb[:],
                    start=False,
                    stop=True,
                )
                nc.scalar.copy(out=zstage[0:1, b * OUT : (b + 1) * OUT], in_=ps[:])
                if b == BL // 2 - 1:
                    # first half of the output can ship while the second half
                    # is still being computed
                    nc.sync.dma_start(
                        z.ap()[:, : (BL // 2) * OUT],
                        zstage[0:1, : (BL // 2) * OUT],
                    )

            nc.sync.dma_start(
                z.ap()[:, (BL // 2) * OUT :], zstage[0:1, (BL // 2) * OUT :]
            )

    nc.compile()
    return nc


def get_nc(pool_entries=POOL):
    key = ("nc", pool_entries)
    if key not in _CACHE:
        _CACHE[key] = _build(pool_entries)
    return _CACHE[key]


def make_in_maps(X, W, bias, Werr, Berr, loc_id):
    X = np.ascontiguousarray(np.asarray(X, dtype=np.float32))
    W = np.ascontiguousarray(np.asarray(W, dtype=np.float32))
    bias = np.ascontiguousarray(np.asarray(bias, dtype=np.float32))
    Werr = np.ascontiguousarray(np.asarray(Werr, dtype=np.float32))
    Berr = np.ascontiguousarray(np.asarray(Berr, dtype=np.float32))
    loc_id = np.ascontiguousarray(np.asarray(loc_id, dtype=np.int32))

    pool_entries = Werr.shape[0]
    werr2d = Werr.reshape(pool_entries * 128, WT_COLS)
    wr = W.reshape(128, WT_COLS)
    bias16 = np.ascontiguousarray(np.broadcast_to(bias[None, :], (BL, OUT)))
    eye16 = np.eye(BL, dtype=np.float32)
    p_iota = np.arange(128, dtype=np.int32)[:, None]

    in_maps = []
    for c in range(NCORES):
        xc = X[c * BL : (c + 1) * BL]  # [BL, IN]
        locc = loc_id[c * BL : (c + 1) * BL]  # [BL]
        xt = np.ascontiguousarray(
            xc.reshape(BL, 128, 4).transpose(1, 0, 2).reshape(128, BL * 4)
        )
        idxc = np.ascontiguousarray(locc[None, :] * 128 + p_iota).astype(np.int32)
        in_maps.append(
            {
                "Werr": werr2d,
                "Berr": Berr,
                "Wr": wr,
                "Xt": xt,
                "idx": idxc,
                "loc": np.ascontiguousarray(locc[:, None]),
                "bias16": bias16,
                "eye16": eye16,
            }
        )
    return in_maps


def _reset_accelerator():
    import ctypes

    try:
        lib = ctypes.CDLL("/opt/axon/libaxon_pjrt.so")
        lib.axon_reset.restype = ctypes.c_int64
        lib.axon_reset()
    except Exception:
        pass


def kernel(X, W, bias, Werr, Berr, loc_id):
    from concourse.bass_utils import run_bass_kernel_spmd

    nc = get_nc()
    in_maps = make_in_maps(X, W, bias, Werr, Berr, loc_id)
    try:
        res = run_bass_kernel_spmd(nc, in_maps, core_ids=list(range(NCORES)))
    except Exception:
        # a wedged NeuronCore surfaces as an unrecoverable-device error;
        # reset the accelerator once and retry
        _reset_accelerator()
        res = run_bass_kernel_spmd(nc, in_maps, core_ids=list(range(NCORES)))
    out = np.concatenate(
        [res.results[c]["Z"].reshape(BL, OUT) for c in range(NCORES)], axis=0
    )
    return out


# revision 4
# speedup vs baseline: 1.1560x; 1.1560x over previous
"""Bass/Trainium2 kernel for DropConnect (training path, Wstd != 0).

Z[b,o] = sum_i X[b,i] * W[i,o] * Werr[loc_id[b],i,o] + bias[o] * Berr[loc_id[b],o]

Strategy (8 NeuronCores, data-parallel over batch):
  - each core handles 16 samples; W and the Werr pool are replicated.
    bias*Berr[loc] is precomputed on the host (loc_id is host-visible) and
    shipped as a [16, 512] input, so no Berr gather happens on device.
  - per sample, the 1MB Werr[loc] slab is gathered on-device with one indirect
    DMA that CASTS f32 -> bf16 in flight (software-DGE feature): Werr viewed
    as [128000, 2048] macro-rows, dest partition p pulls the contiguous 8KB
    macro-row loc*128+p (input rows i=4p..4p+3). All 16 gathers are issued
    back-to-back upfront on gpsimd so the 16 DMA engines never starve.
  - W / Xt / eye / membias are also loaded via gpsimd cast-DMAs into bf16.
  - VectorE computes the bf16 W*Werr product per sample ([128,2048] 16-bit
    tensor_tensor, 2x DVE rate; halved SBUF traffic keeps PE streaming fast)
  - TensorE contracts with X: 4 matmuls of [128,1]x[128,512] bf16 into a
    [1,512] PSUM tile, plus a 5th matmul (one-hot [16,1] against the
    membias [16,512] bf16 tile) to add the bias
  - ScalarE copies each sample's PSUM row into a [1, 8192] staging tile;
    output ships in two halves
"""

import sys

sys.path.insert(0, "/opt/trn_rl_repo")

import numpy as np

B, IN, OUT, POOL, NCORES = 128, 512, 512, 1000, 8
BL = B // NCORES  # samples per core
WT_COLS = 4 * OUT  # 2048: one macro-row = 4 input rows of W/Werr

_CACHE = {}


def _build(pool_entries=POOL):
    import concourse.bass as bass
    import concourse.mybir as mybir
    import concourse.tile as tile
    from concourse import bacc

    f32, i32, bf16 = mybir.dt.float32, mybir.dt.int32, mybir.dt.bfloat16

    nc = bacc.Bacc("TRN2", debug=False)
    werr = nc.dram_tensor(
        "Werr", [pool_entries * 128, WT_COLS], f32, kind="ExternalInput"
    )
    wr = nc.dram_tensor("Wr", [128, WT_COLS], f32, kind="ExternalInput")
    xt = nc.dram_tensor("Xt", [128, BL * 4], f32, kind="ExternalInput")
    idx = nc.dram_tensor("idx", [128, BL], i32, kind="ExternalInput")
    memb = nc.dram_tensor("memb", [BL, OUT], f32, kind="ExternalInput")
    eye16 = nc.dram_tensor("eye16", [BL, BL], f32, kind="ExternalInput")
    z = nc.dram_tensor("Z", [1, BL * OUT], f32, kind="ExternalOutput")

    with tile.TileContext(nc) as tc:
        with (
            tc.tile_pool(name="const", bufs=1) as cpool,
            tc.tile_pool(name="wts", bufs=BL) as wpool,
            tc.tile_pool(name="prod", bufs=4) as ptpool,
            tc.tile_pool(name="ps", bufs=8, space="PSUM") as ppool,
        ):
            # idx first: the Werr gathers are gated only on this tiny load
            idx_sb = cpool.tile([128, BL], i32)
            nc.sync.dma_start(idx_sb[:], idx.ap())

            # small bf16 cast-loads on the gpsimd software DGE
            wr_sb = cpool.tile([128, WT_COLS], bf16)
            nc.gpsimd.dma_start(out=wr_sb[:], in_=wr.ap())
            xt_sb = cpool.tile([128, BL * 4], bf16)
            nc.gpsimd.dma_start(out=xt_sb[:], in_=xt.ap())
            eye_sb = cpool.tile([BL, BL], bf16)
            nc.gpsimd.dma_start(out=eye_sb[:], in_=eye16.ap())
            memb_sb = cpool.tile([BL, OUT], bf16)
            nc.gpsimd.dma_start(out=memb_sb[:], in_=memb.ap())
            zstage = cpool.tile([1, BL * OUT], f32)

            # all 16 slab gathers enqueue back-to-back on gpsimd, casting
            # f32 -> bf16 in flight
            wts = []
            for b in range(BL):
                wt = wpool.tile([128, WT_COLS], bf16, tag="wt")
                nc.gpsimd.indirect_dma_start(
                    out=wt[:],
                    out_offset=None,
                    in_=werr.ap(),
                    in_offset=bass.IndirectOffsetOnAxis(
                        ap=idx_sb[:, b : b + 1], axis=0
                    ),
                )
                wts.append(wt)

            for b in range(BL):
                wt = wts[b]
                pt = ptpool.tile([128, WT_COLS], bf16, tag="pt")
                nc.vector.tensor_mul(pt[:], wt[:], wr_sb[:])
                ps = ppool.tile([1, OUT], f32, tag="ps")
                for j in range(4):
                    nc.tensor.matmul(
                        out=ps[:],
                        lhsT=xt_sb[:, 4 * b + j : 4 * b + j + 1],
                        rhs=pt[:, j * OUT : (j + 1) * OUT],
                        start=(j == 0),
                        stop=False,
                    )
                nc.tensor.matmul(
                    out=ps[:],
                    lhsT=eye_sb[:, b : b + 1],
                    rhs=memb_sb[:],
                    start=False,
                    stop=True,
                )
                nc.scalar.copy(out=zstage[0:1, b * OUT : (b + 1) * OUT], in_=ps[:])
                if b == BL // 2 - 1:
                    # first half of the output can ship while the second half
                    # is still being computed
                    nc.sync.dma_start(
                        z.ap()[:, : (BL // 2) * OUT],
                        zstage[0:1, : (BL // 2) * OUT],
                    )

            nc.sync.dma_start(
                z.ap()[:, (BL // 2) * OUT :], zstage[0:1, (BL // 2) * OUT :]
            )

    nc.compile()
    return nc


def get_nc(pool_entries=POOL):
    key = ("nc", pool_entries)
    if key not in _CACHE:
        _CACHE[key] = _build(pool_entries)
    return _CACHE[key]


def make_in_maps(X, W, bias, Werr, Berr, loc_id):
    X = np.ascontiguousarray(np.asarray(X, dtype=np.float32))
    W = np.ascontiguousarray(np.asarray(W, dtype=np.float32))
    bias = np.ascontiguousarray(np.asarray(bias, dtype=np.float32))
    Werr = np.ascontiguousarray(np.asarray(Werr, dtype=np.float32))
    Berr = np.ascontiguousarray(np.asarray(Berr, dtype=np.float32))
    loc_id = np.ascontiguousarray(np.asarray(loc_id, dtype=np.int32))

    pool_entries = Werr.shape[0]
    werr2d = Werr.reshape(pool_entries * 128, WT_COLS)
    wr = W.reshape(128, WT_COLS)
    eye16 = np.eye(BL, dtype=np.float32)
    p_iota = np.arange(128, dtype=np.int32)[:, None]

    in_maps = []
    for c in range(NCORES):
        xc = X[c * BL : (c + 1) * BL]  # [BL, IN]
        locc = loc_id[c * BL : (c + 1) * BL]  # [BL]
        xt = np.ascontiguousarray(
            xc.reshape(BL, 128, 4).transpose(1, 0, 2).reshape(128, BL * 4)
        )
        idxc = np.ascontiguousarray(locc[None, :] * 128 + p_iota).astype(np.int32)
        membc = np.ascontiguousarray(bias[None, :] * Berr[locc])  # [BL, OUT]
        in_maps.append(
            {
                "Werr": werr2d,
                "Wr": wr,
                "Xt": xt,
                "idx": idxc,
                "memb": membc,
                "eye16": eye16,
            }
        )
    return in_maps


def _reset_accelerator():
    import ctypes

    try:
        lib = ctypes.CDLL("/opt/axon/libaxon_pjrt.so")
        lib.axon_reset.restype = ctypes.c_int64
        lib.axon_reset()
    except Exception:
        pass


def kernel(X, W, bias, Werr, Berr, loc_id):
    from concourse.bass_utils import run_bass_kernel_spmd

    nc = get_nc()
    in_maps = make_in_maps(X, W, bias, Werr, Berr, loc_id)
    try:
        res = run_bass_kernel_spmd(nc, in_maps, core_ids=list(range(NCORES)))
    except Exception:
        # a wedged NeuronCore surfaces as an unrecoverable-device error;
        # reset the accelerator once and retry
        _reset_accelerator()
        res = run_bass_kernel_spmd(nc, in_maps, core_ids=list(range(NCORES)))
    out = np.concatenate(
        [res.results[c]["Z"].reshape(BL, OUT) for c in range(NCORES)], axis=0
    )
    return out


# revision 5
# speedup vs baseline: 1.1698x; 1.0120x over previous
"""Bass/Trainium2 kernel for DropConnect (training path, Wstd != 0).

Z[b,o] = sum_i X[b,i] * W[i,o] * Werr[loc_id[b],i,o] + bias[o] * Berr[loc_id[b],o]

Strategy (8 NeuronCores, data-parallel over batch):
  - each core handles 16 samples; W and the Werr pool are replicated.
    bias*Berr[loc] is precomputed on the host (loc_id is host-visible) and
    shipped as a flat [1, 16*512] f32 row, so no Berr gather and no bias
    matmul happen on device.
  - per sample, the 1MB Werr[loc] slab is gathered on-device with one indirect
    DMA that CASTS f32 -> bf16 in flight (software-DGE feature): Werr viewed
    as [128000, 2048] macro-rows, dest partition p pulls the contiguous 8KB
    macro-row loc*128+p (input rows i=4p..4p+3). All 16 gathers are issued
    back-to-back upfront on gpsimd so the 16 DMA engines never starve; the
    first four go ahead of the small cast-loads to start the pipeline early.
  - VectorE computes the bf16 W*Werr product per sample ([128,2048] 16-bit
    tensor_tensor at 2x DVE rate), and also evicts each sample's PSUM row
    with a fused tensor_add against the membias row (bias add + eviction in
    one [1,512] op), emitted with a one-sample lag so the adds don't
    serialize the TT pipeline against TensorE.
  - TensorE contracts with X: 4 matmuls of [128,1]x[128,512] bf16 into a
    [1,512] PSUM tile per sample. Output ships in two halves.
"""

import sys

sys.path.insert(0, "/opt/trn_rl_repo")

import numpy as np

B, IN, OUT, POOL, NCORES = 128, 512, 512, 1000, 8
BL = B // NCORES  # samples per core
WT_COLS = 4 * OUT  # 2048: one macro-row = 4 input rows of W/Werr

_CACHE = {}


def _build(pool_entries=POOL):
    import concourse.bass as bass
    import concourse.mybir as mybir
    import concourse.tile as tile
    from concourse import bacc

    f32, i32, bf16 = mybir.dt.float32, mybir.dt.int32, mybir.dt.bfloat16

    nc = bacc.Bacc("TRN2", debug=False)
    werr = nc.dram_tensor(
        "Werr", [pool_entries * 128, WT_COLS], f32, kind="ExternalInput"
    )
    wr = nc.dram_tensor("Wr", [128, WT_COLS], f32, kind="ExternalInput")
    xt = nc.dram_tensor("Xt", [128, BL * 4], f32, kind="ExternalInput")
    idx = nc.dram_tensor("idx", [128, BL], i32, kind="ExternalInput")
    memb = nc.dram_tensor("memb", [1, BL * OUT], f32, kind="ExternalInput")
    z = nc.dram_tensor("Z", [1, BL * OUT], f32, kind="ExternalOutput")

    EARLY = 4  # gathers issued before the small cast-loads

    with tile.TileContext(nc) as tc:
        with (
            tc.tile_pool(name="const", bufs=1) as cpool,
            tc.tile_pool(name="wts", bufs=BL) as wpool,
            tc.tile_pool(name="prod", bufs=4) as ptpool,
            tc.tile_pool(name="ps", bufs=8, space="PSUM") as ppool,
        ):
            # idx first: the Werr gathers are gated only on this tiny load
            idx_sb = cpool.tile([128, BL], i32)
            nc.sync.dma_start(idx_sb[:], idx.ap())
            memb_sb = cpool.tile([1, BL * OUT], f32)
            nc.sync.dma_start(memb_sb[:], memb.ap())

            def gather(b):
                wt = wpool.tile([128, WT_COLS], bf16, tag="wt")
                nc.gpsimd.indirect_dma_start(
                    out=wt[:],
                    out_offset=None,
                    in_=werr.ap(),
                    in_offset=bass.IndirectOffsetOnAxis(
                        ap=idx_sb[:, b : b + 1], axis=0
                    ),
                )
                return wt

            wts = [gather(b) for b in range(EARLY)]

            # small bf16 cast-loads on the gpsimd software DGE
            wr_sb = cpool.tile([128, WT_COLS], bf16)
            nc.gpsimd.dma_start(out=wr_sb[:], in_=wr.ap())
            xt_sb = cpool.tile([128, BL * 4], bf16)
            nc.gpsimd.dma_start(out=xt_sb[:], in_=xt.ap())
            zstage = cpool.tile([1, BL * OUT], f32)

            wts += [gather(b) for b in range(EARLY, BL)]

            prev = None  # (ps tile, sample index) awaiting eviction
            for b in range(BL):
                wt = wts[b]
                pt = ptpool.tile([128, WT_COLS], bf16, tag="pt")
                nc.vector.tensor_mul(pt[:], wt[:], wr_sb[:])
                ps = ppool.tile([1, OUT], f32, tag="ps")
                for j in range(4):
                    nc.tensor.matmul(
                        out=ps[:],
                        lhsT=xt_sb[:, 4 * b + j : 4 * b + j + 1],
                        rhs=pt[:, j * OUT : (j + 1) * OUT],
                        start=(j == 0),
                        stop=(j == 3),
                    )
                if prev is not None:
                    pb = prev[1]
                    nc.vector.tensor_add(
                        zstage[0:1, pb * OUT : (pb + 1) * OUT],
                        prev[0][:],
                        memb_sb[0:1, pb * OUT : (pb + 1) * OUT],
                    )
                    if pb == BL // 2 - 1:
                        # first half of the output ships while the second
                        # half is still being computed
                        nc.sync.dma_start(
                            z.ap()[:, : (BL // 2) * OUT],
                            zstage[0:1, : (BL // 2) * OUT],
                        )
                prev = (ps, b)

            pb = prev[1]
            nc.vector.tensor_add(
                zstage[0:1, pb * OUT : (pb + 1) * OUT],
                prev[0][:],
                memb_sb[0:1, pb * OUT : (pb + 1) * OUT],
            )
            nc.sync.dma_start(
                z.ap()[:, (BL // 2) * OUT :], zstage[0:1, (BL // 2) * OUT :]
            )

    nc.compile()
    return nc


def get_nc(pool_entries=POOL):
    key = ("nc", pool_entries)
    if key not in _CACHE:
        _CACHE[key] = _build(pool_entries)
    return _CACHE[key]


def make_in_maps(X, W, bias, Werr, Berr, loc_id):
    X = np.ascontiguousarray(np.asarray(X, dtype=np.float32))
    W = np.ascontiguousarray(np.asarray(W, dtype=np.float32))
    bias = np.ascontiguousarray(np.asarray(bias, dtype=np.float32))
    Werr = np.ascontiguousarray(np.asarray(Werr, dtype=np.float32))
    Berr = np.ascontiguousarray(np.asarray(Berr, dtype=np.float32))
    loc_id = np.ascontiguousarray(np.asarray(loc_id, dtype=np.int32))

    pool_entries = Werr.shape[0]
    werr2d = Werr.reshape(pool_entries * 128, WT_COLS)
    wr = W.reshape(128, WT_COLS)
    p_iota = np.arange(128, dtype=np.int32)[:, None]

    in_maps = []
    for c in range(NCORES):
        xc = X[c * BL : (c + 1) * BL]  # [BL, IN]
        locc = loc_id[c * BL : (c + 1) * BL]  # [BL]
        xt = np.ascontiguousarray(
            xc.reshape(BL, 128, 4).transpose(1, 0, 2).reshape(128, BL * 4)
        )
        idxc = np.ascontiguousarray(locc[None, :] * 128 + p_iota).astype(np.int32)
        membc = np.ascontiguousarray(
            (bias[None, :] * Berr[locc]).reshape(1, BL * OUT)
        )
        in_maps.append(
            {
                "Werr": werr2d,
                "Wr": wr,
                "Xt": xt,
                "idx": idxc,
                "memb": membc,
            }
        )
    return in_maps


def _reset_accelerator():
    import ctypes

    try:
        lib = ctypes.CDLL("/opt/axon/libaxon_pjrt.so")
        lib.axon_reset.restype = ctypes.c_int64
        lib.axon_reset()
    except Exception:
        pass


def kernel(X, W, bias, Werr, Berr, loc_id):
    from concourse.bass_utils import run_bass_kernel_spmd

    nc = get_nc()
    in_maps = make_in_maps(X, W, bias, Werr, Berr, loc_id)
    try:
        res = run_bass_kernel_spmd(nc, in_maps, core_ids=list(range(NCORES)))
    except Exception:
        # a wedged NeuronCore surfaces as an unrecoverable-device error;
        # reset the accelerator once and retry
        _reset_accelerator()
        res = run_bass_kernel_spmd(nc, in_maps, core_ids=list(range(NCORES)))
    out = np.concatenate(
        [res.results[c]["Z"].reshape(BL, OUT) for c in range(NCORES)], axis=0
    )
    return out


# revision 8
# speedup vs baseline: 1.1736x; 1.0033x over previous
"""Bass/Trainium2 kernel for DropConnect (training path, Wstd != 0).

Z[b,o] = sum_i X[b,i] * W[i,o] * Werr[loc_id[b],i,o] + bias[o] * Berr[loc_id[b],o]

Strategy (8 NeuronCores, data-parallel over batch):
  - each core handles 16 samples; W and the Werr pool are replicated.
    bias*Berr[loc] is precomputed on the host (loc_id is host-visible) and
    shipped as a flat [1, 16*512] f32 row, so no Berr gather and no bias
    matmul happen on device.
  - per sample, the 1MB Werr[loc] slab is gathered on-device with one indirect
    DMA that CASTS f32 -> bf16 in flight (software-DGE feature): Werr viewed
    as [128000, 2048] macro-rows, dest partition p pulls the contiguous 8KB
    macro-row loc*128+p (input rows i=4p..4p+3). All 16 gathers are issued
    back-to-back upfront on gpsimd so the 16 DMA engines never starve; the
    first four go ahead of the small cast-loads to start the pipeline early.
  - VectorE computes the bf16 W*Werr product per sample ([128,2048] 16-bit
    tensor_tensor at 2x DVE rate), and also evicts each sample's PSUM row
    with a fused tensor_add against the membias row (bias add + eviction in
    one [1,512] op), emitted with a one-sample lag so the adds don't
    serialize the TT pipeline against TensorE.
  - TensorE contracts with X: 4 matmuls of [128,1]x[128,512] bf16 into a
    [1,512] PSUM tile per sample. Output ships in two halves.
"""

import sys

sys.path.insert(0, "/opt/trn_rl_repo")

import numpy as np

B, IN, OUT, POOL, NCORES = 128, 512, 512, 1000, 8
BL = B // NCORES  # samples per core
WT_COLS = 4 * OUT  # 2048: one macro-row = 4 input rows of W/Werr

_CACHE = {}


def _build(pool_entries=POOL):
    import concourse.bass as bass
    import concourse.mybir as mybir
    import concourse.tile as tile
    from concourse import bacc

    f32, i32, bf16 = mybir.dt.float32, mybir.dt.int32, mybir.dt.bfloat16

    nc = bacc.Bacc("TRN2", debug=False)
    werr = nc.dram_tensor(
        "Werr", [pool_entries * 128, WT_COLS], f32, kind="ExternalInput"
    )
    wr = nc.dram_tensor("Wr", [128, WT_COLS], f32, kind="ExternalInput")
    xt = nc.dram_tensor("Xt", [128, BL * 4], f32, kind="ExternalInput")
    idx = nc.dram_tensor("idx", [128, BL], i32, kind="ExternalInput")
    memb = nc.dram_tensor("memb", [1, BL * OUT], f32, kind="ExternalInput")
    z = nc.dram_tensor("Z", [1, BL * OUT], f32, kind="ExternalOutput")

    EARLY = 4  # gathers issued before the small cast-loads

    with tile.TileContext(nc) as tc:
        with (
            tc.tile_pool(name="const", bufs=1) as cpool,
            tc.tile_pool(name="wts", bufs=BL) as wpool,
            tc.tile_pool(name="prod", bufs=4) as ptpool,
            tc.tile_pool(name="ps", bufs=8, space="PSUM") as ppool,
        ):
            # idx first: the Werr gathers are gated only on this tiny load
            idx_sb = cpool.tile([128, BL], i32)
            nc.sync.dma_start(idx_sb[:], idx.ap())
            memb_sb = cpool.tile([1, BL * OUT], f32)
            nc.sync.dma_start(memb_sb[:], memb.ap())

            def gather(b):
                wt = wpool.tile([128, WT_COLS], bf16, tag="wt")
                nc.gpsimd.indirect_dma_start(
                    out=wt[:],
                    out_offset=None,
                    in_=werr.ap(),
                    in_offset=bass.IndirectOffsetOnAxis(
                        ap=idx_sb[:, b : b + 1], axis=0
                    ),
                )
                return wt

            wts = [gather(b) for b in range(EARLY)]

            # small bf16 cast-loads on the gpsimd software DGE
            wr_sb = cpool.tile([128, WT_COLS], bf16)
            nc.gpsimd.dma_start(out=wr_sb[:], in_=wr.ap())
            xt_sb = cpool.tile([128, BL * 4], bf16)
            nc.gpsimd.dma_start(out=xt_sb[:], in_=xt.ap())
            zstage = cpool.tile([1, BL * OUT], f32)

            wts += [gather(b) for b in range(EARLY, BL)]

            prev = None  # (ps tile, sample index) awaiting eviction
            for b in range(BL):
                wt = wts[b]
                pt = ptpool.tile([128, WT_COLS], bf16, tag="pt")
                nc.vector.tensor_mul(pt[:], wt[:], wr_sb[:])
                ps = ppool.tile([1, OUT], f32, tag="ps")
                for j in range(4):
                    nc.tensor.matmul(
                        out=ps[:],
                        lhsT=xt_sb[:, 4 * b + j : 4 * b + j + 1],
                        rhs=pt[:, j * OUT : (j + 1) * OUT],
                        start=(j == 0),
                        stop=(j == 3),
                    )
                if prev is not None:
                    pb = prev[1]
                    nc.vector.tensor_add(
                        zstage[0:1, pb * OUT : (pb + 1) * OUT],
                        prev[0][:],
                        memb_sb[0:1, pb * OUT : (pb + 1) * OUT],
                    )
                    if pb == BL // 2 - 1:
                        # first half of the output ships while the second
                        # half is still being computed
                        nc.sync.dma_start(
                            z.ap()[:, : (BL // 2) * OUT],
                            zstage[0:1, : (BL // 2) * OUT],
                        )
                prev = (ps, b)

            pb = prev[1]
            nc.vector.tensor_add(
                zstage[0:1, pb * OUT : (pb + 1) * OUT],
                prev[0][:],
                memb_sb[0:1, pb * OUT : (pb + 1) * OUT],
            )
            nc.sync.dma_start(
                z.ap()[:, (BL // 2) * OUT :], zstage[0:1, (BL // 2) * OUT :]
            )

    nc.compile()
    return nc


def get_nc(pool_entries=POOL):
    key = ("nc", pool_entries)
    if key not in _CACHE:
        _CACHE[key] = _build(pool_entries)
    return _CACHE[key]


def make_in_maps(X, W, bias, Werr, Berr, loc_id):
    X = np.ascontiguousarray(np.asarray(X, dtype=np.float32))
    W = np.ascontiguousarray(np.asarray(W, dtype=np.float32))
    bias = np.ascontiguousarray(np.asarray(bias, dtype=np.float32))
    Werr = np.ascontiguousarray(np.asarray(Werr, dtype=np.float32))
    Berr = np.ascontiguousarray(np.asarray(Berr, dtype=np.float32))
    loc_id = np.ascontiguousarray(np.asarray(loc_id, dtype=np.int32))

    pool_entries = Werr.shape[0]
    werr2d = Werr.reshape(pool_entries * 128, WT_COLS)
    wr = W.reshape(128, WT_COLS)
    p_iota = np.arange(128, dtype=np.int32)[:, None]

    in_maps = []
    for c in range(NCORES):
        xc = X[c * BL : (c + 1) * BL]  # [BL, IN]
        locc = loc_id[c * BL : (c + 1) * BL]  # [BL]
        xt = np.ascontiguousarray(
            xc.reshape(BL, 128, 4).transpose(1, 0, 2).reshape(128, BL * 4)
        )
        idxc = np.ascontiguousarray(locc[None, :] * 128 + p_iota).astype(np.int32)
        membc = np.ascontiguousarray(
            (bias[None, :] * Berr[locc]).reshape(1, BL * OUT)
        )
        in_maps.append(
            {
                "Werr": werr2d,
                "Wr": wr,
                "Xt": xt,
                "idx": idxc,
                "memb": membc,
            }
        )
    return in_maps


def _reset_accelerator():
    import ctypes

    try:
        lib = ctypes.CDLL("/opt/axon/libaxon_pjrt.so")
        lib.axon_reset.restype = ctypes.c_int64
        lib.axon_reset()
    except Exception:
        pass


def kernel(X, W, bias, Werr, Berr, loc_id):
    from concourse.bass_utils import run_bass_kernel_spmd

    nc = get_nc()
    in_maps = make_in_maps(X, W, bias, Werr, Berr, loc_id)
    try:
        res = run_bass_kernel_spmd(nc, in_maps, core_ids=list(range(NCORES)))
    except Exception:
        # a wedged NeuronCore surfaces as an unrecoverable-device error;
        # reset the accelerator once and retry
        _reset_accelerator()
        res = run_bass_kernel_spmd(nc, in_maps, core_ids=list(range(NCORES)))
    out = np.concatenate(
        [res.results[c]["Z"].reshape(BL, OUT) for c in range(NCORES)], axis=0
    )
    return out
